# revision 2
# baseline (speedup 1.0000x reference)
"""HeatmapRegressionHead Trainium2 kernel.

Data-parallel over batch: 32 images -> 8 NeuronCores x 4 images.
Per core: 5 conv layers (3x3 via 9 shifted fp32r matmuls accumulating in PSUM,
BN folded into weights host-side) + 3 1x1 heads + on-device decode
(soft-argmax, masked-window refinement, bilinear offset sampling via
separable tent weights).

Self-contained: only needs numpy + concourse (+ jax via bass_utils paths).
"""
import numpy as np

import concourse.bass as bass
import concourse.mybir as mybir
import concourse.tile as tile
from concourse import bacc
from concourse.bass_utils import run_bass_kernel_spmd

F32 = mybir.dt.float32
F32R = mybir.dt.float32r
AF = mybir.ActivationFunctionType
ALU = mybir.AluOpType
AX = mybir.AxisListType

B, C, H, W = 32, 256, 64, 64
HID, K = 256, 17
NCORE = 8
BL = B // NCORE          # images per core
HP, WP = H + 2, W + 2    # padded spatial
PADSZ = HP * WP          # 4356
NBLK = 8                 # spatial blocks per image (8 rows x 64 cols = 512)
BLKF = H * W // NBLK     # 512 free elements per block
BN_EPS = 1e-5


def tf32_round(a):
    u = np.ascontiguousarray(a, dtype=np.float32).view(np.uint32)
    r = u + np.uint32(0x00000FFF) + ((u >> np.uint32(13)) & np.uint32(1))
    r &= np.uint32(0xFFFFE000)
    return r.view(np.float32)


def _pack_conv3(w, scale):
    """(Cout, Cin, 3, 3) [* scale per Cout] -> (ncin_t, 128, 9 * ncout_t * 128)."""
    cout, cin = w.shape[0], w.shape[1]
    ncin_t = cin // 128
    assert cout % 128 == 0
    wf = w * scale[:, None, None, None]
    arr = wf.transpose(1, 2, 3, 0).reshape(ncin_t, 128, 9 * cout)
    return tf32_round(np.ascontiguousarray(arr, dtype=np.float32))


def _pack_conv1(w):
    """(Cout, Cin, 1, 1) -> (ncin_t, 128, Cout)."""
    cout, cin = w.shape[0], w.shape[1]
    ncin_t = cin // 128
    arr = w[:, :, 0, 0].T.reshape(ncin_t, 128, cout)
    return tf32_round(np.ascontiguousarray(arr, dtype=np.float32))


def build_nc(a_sig: float, fw_sig: float, repeat: int = 1):
    """Build the per-core bass program. All cores run the same program (SPMD)."""
    nc = bacc.Bacc()

    P = {}
    P["xin"] = nc.declare_dram_parameter("xin", [BL, C, H * W], F32R, isOutput=False)
    for nm in ("w_s1", "w_s2", "w_h1", "w_o1"):
        P[nm] = nc.declare_dram_parameter(nm, [2, 128, 9 * 2 * 128], F32R, isOutput=False)
    P["w_v1"] = nc.declare_dram_parameter("w_v1", [2, 128, 9 * 128], F32R, isOutput=False)
    P["w_h2"] = nc.declare_dram_parameter("w_h2", [2, 128, K], F32R, isOutput=False)
    P["w_o2"] = nc.declare_dram_parameter("w_o2", [2, 128, 2 * K], F32R, isOutput=False)
    P["w_v2"] = nc.declare_dram_parameter("w_v2", [1, 128, K], F32R, isOutput=False)
    # bias4 cols: s1_t0, s1_t1, s2_t0, s2_t1, h1_t0, h1_t1, o1_t0, o1_t1
    P["bias4"] = nc.declare_dram_parameter("bias4", [128, 8], F32, isOutput=False)
    P["biasv1"] = nc.declare_dram_parameter("biasv1", [128, 1], F32, isOutput=False)
    # bias1x1 cols: c_h2 (rows 0:17), c_o2 reordered (rows 0:34), c_v2 (rows 0:17)
    P["bias1x1"] = nc.declare_dram_parameter("bias1x1", [2 * K, 3], F32, isOutput=False)
    P["grid"] = nc.declare_dram_parameter("grid", [128, 64], F32, isOutput=False)

    P["heat_o"] = nc.declare_dram_parameter("heat_o", [BL, K, H, W], F32, isOutput=True)
    P["off_o"] = nc.declare_dram_parameter("off_o", [BL, K, 2, H, W], F32, isOutput=True)
    P["var_o"] = nc.declare_dram_parameter("var_o", [BL, K, H, W], F32, isOutput=True)
    P["coords_o"] = nc.declare_dram_parameter("coords_o", [128, 2], F32, isOutput=True)
    P["scores_o"] = nc.declare_dram_parameter("scores_o", [128, 1], F32, isOutput=True)

    with tile.TileContext(nc) as tc:
        if repeat == 1:
            _body(nc, tc, P, a_sig, fw_sig)
        else:
            with tc.For_i(0, repeat, 1):
                _body(nc, tc, P, a_sig, fw_sig)
    return nc


def _body(nc, tc, P, a_sig, fw_sig):
    _conv_phase(nc, tc, P)
    _decode_phase(nc, tc, P, a_sig, fw_sig)


def _conv_phase(nc, tc, P):
    import contextlib
    with contextlib.ExitStack() as ctx:
        sbw = ctx.enter_context(tc.tile_pool(name="sbw", bufs=2))   # conv weights (stream)
        sbp = ctx.enter_context(tc.tile_pool(name="sbp", bufs=1))   # persistent
        sbb = ctx.enter_context(tc.tile_pool(name="sbb", bufs=3))   # branch blocks
        sbs = ctx.enter_context(tc.tile_pool(name="sbs", bufs=4))   # f32 staging
        ps = ctx.enter_context(tc.tile_pool(name="ps", bufs=8, space="PSUM"))

        xpad = [sbp.tile([128, PADSZ], F32R, name=f"xpad{i}", tag=f"xpad{i}") for i in range(2)]
        s1pad = [sbp.tile([128, PADSZ], F32R, name=f"s1pad{i}", tag=f"s1pad{i}") for i in range(2)]
        spad = [sbp.tile([128, PADSZ], F32R, name=f"spad{i}", tag=f"spad{i}") for i in range(2)]
        h0full = sbp.tile([128, H * W], F32R, name="h0full", tag="h0full")
        o0full = sbp.tile([128, H * W], F32R, name="o0full", tag="o0full")
        b4 = sbp.tile([128, 8], F32, name="b4", tag="b4")
        bv1 = sbp.tile([128, 1], F32, name="bv1", tag="bv1")
        b1x1 = sbp.tile([2 * K, 3], F32, name="b1x1", tag="b1x1")
        wh2 = sbp.tile([128, 2 * K], F32R, name="wh2", tag="wh2")
        wo2 = sbp.tile([128, 2 * 2 * K], F32R, name="wo2", tag="wo2")
        wv2 = sbp.tile([128, K], F32R, name="wv2", tag="wv2")
        zf = sbp.tile([128, 1], F32, name="zf", tag="zf")

        nc.vector.memset(zf[:], 0.0)
        nc.sync.dma_start(b4[:], P["bias4"][:])
        nc.sync.dma_start(bv1[:], P["biasv1"][:])
        nc.sync.dma_start(b1x1[:], P["bias1x1"][:])
        nc.sync.dma_start(wh2[:, 0:K], P["w_h2"][0])
        nc.sync.dma_start(wh2[:, K:2 * K], P["w_h2"][1])
        nc.sync.dma_start(wo2[:, 0:2 * K], P["w_o2"][0])
        nc.sync.dma_start(wo2[:, 2 * K:4 * K], P["w_o2"][1])
        nc.sync.dma_start(wv2[:], P["w_v2"][0])

        # zero padded tiles once (pads stay zero; interiors get overwritten)
        for t in (*xpad, *s1pad, *spad):
            nc.vector.tensor_copy(t[:], zf[:].broadcast_to([128, PADSZ]))

        def pv(t):
            return t[:].rearrange("p (h w) -> p h w", h=HP)

        def conv3(img, wdram, wname, src_tiles, ncout_t, evac):
            wt = sbw.tile([128, 2 * 9 * ncout_t * 128], F32R,
                          name=f"w_{wname}_{img}", tag="wconv")
            nc.sync.dma_start(wt[:, 0:9 * ncout_t * 128], wdram[0])
            nc.sync.dma_start(wt[:, 9 * ncout_t * 128:], wdram[1])
            for ct in range(ncout_t):
                pts = []
                for blk in range(NBLK):
                    pt = ps.tile([128, BLKF], F32,
                                 name=f"ps_{wname}_{img}_{ct}_{blk}", tag="psum")
                    pts.append(pt)
                for ci in range(2):
                    srcv = pv(src_tiles[ci])
                    for o in range(9):
                        dy, dx = o // 3, o % 3
                        base = (ci * 9 * ncout_t + o * ncout_t + ct) * 128
                        wsl = wt[:, base:base + 128]
                        k = ci * 9 + o
                        for blk in range(NBLK):
                            rhs = srcv[:, blk * 8 + dy: blk * 8 + dy + 8, dx: dx + 64]
                            nc.tensor.matmul(pts[blk][:], wsl, rhs,
                                             start=(k == 0), stop=(k == 17))
                for blk in range(NBLK):
                    evac(ct, blk, pts[blk])

        for img in range(BL):
            for ci in range(2):
                nc.sync.dma_start(pv(xpad[ci])[:, 1:1 + H, 1:1 + W],
                                  P["xin"][img, ci * 128:(ci + 1) * 128, :])

            def evac_s1(ct, blk, pt):
                dst = pv(s1pad[ct])[:, blk * 8 + 1: blk * 8 + 9, 1:65]
                nc.scalar.activation(dst, pt[:], AF.Relu, bias=b4[:, ct:ct + 1], scale=1.0)
            conv3(img, P["w_s1"], "s1", xpad, 2, evac_s1)

            def evac_s2(ct, blk, pt):
                dst = pv(spad[ct])[:, blk * 8 + 1: blk * 8 + 9, 1:65]
                nc.scalar.activation(dst, pt[:], AF.Relu, bias=b4[:, 2 + ct:3 + ct], scale=1.0)
            conv3(img, P["w_s2"], "s2", s1pad, 2, evac_s2)

            # ---- h branch ----
            h1blk = {}

            def evac_h1(ct, blk, pt):
                if ct == 0:
                    dst = h0full[:, blk * BLKF:(blk + 1) * BLKF]
                    nc.scalar.activation(dst, pt[:], AF.Relu, bias=b4[:, 4:5], scale=1.0)
                else:
                    t = sbb.tile([128, BLKF], F32R, name=f"h1b_{img}_{blk}", tag="brblk")
                    nc.scalar.activation(t[:], pt[:], AF.Relu, bias=b4[:, 5:6], scale=1.0)
                    h1blk[blk] = t
                    pt2 = ps.tile([128, BLKF], F32, name=f"ps_h2_{img}_{blk}", tag="psum")
                    nc.tensor.matmul(pt2[0:K, :], wh2[:, 0:K],
                                     h0full[:, blk * BLKF:(blk + 1) * BLKF],
                                     start=True, stop=False)
                    nc.tensor.matmul(pt2[0:K, :], wh2[:, K:2 * K], t[:],
                                     start=False, stop=True)
                    st = sbs.tile([128, BLKF], F32, name=f"heat_st_{img}_{blk}", tag="stage")
                    nc.scalar.activation(st[0:K, :], pt2[0:K, :], AF.Identity,
                                         bias=b1x1[0:K, 0:1], scale=1.0)
                    nc.sync.dma_start(P["heat_o"][img, :, blk * 8:(blk + 1) * 8, :],
                                      st[0:K, :])
            conv3(img, P["w_h1"], "h1", spad, 2, evac_h1)

            # ---- o branch ----
            def evac_o1(ct, blk, pt):
                if ct == 0:
                    dst = o0full[:, blk * BLKF:(blk + 1) * BLKF]
                    nc.scalar.activation(dst, pt[:], AF.Relu, bias=b4[:, 6:7], scale=1.0)
                else:
                    t = sbb.tile([128, BLKF], F32R, name=f"o1b_{img}_{blk}", tag="brblk")
                    nc.scalar.activation(t[:], pt[:], AF.Relu, bias=b4[:, 7:8], scale=1.0)
                    pt2 = ps.tile([128, BLKF], F32, name=f"ps_o2_{img}_{blk}", tag="psum")
                    nc.tensor.matmul(pt2[0:2 * K, :], wo2[:, 0:2 * K],
                                     o0full[:, blk * BLKF:(blk + 1) * BLKF],
                                     start=True, stop=False)
                    nc.tensor.matmul(pt2[0:2 * K, :], wo2[:, 2 * K:4 * K], t[:],
                                     start=False, stop=True)
                    st = sbs.tile([128, BLKF], F32, name=f"off_st_{img}_{blk}", tag="stage")
                    nc.scalar.activation(st[0:2 * K, :], pt2[0:2 * K, :], AF.Identity,
                                         bias=b1x1[0:2 * K, 1:2], scale=1.0)
                    nc.sync.dma_start(P["off_o"][img, :, 0, blk * 8:(blk + 1) * 8, :],
                                      st[0:K, :])
                    nc.sync.dma_start(P["off_o"][img, :, 1, blk * 8:(blk + 1) * 8, :],
                                      st[K:2 * K, :])
            conv3(img, P["w_o1"], "o1", spad, 2, evac_o1)

            # ---- v branch ----
            def evac_v1(ct, blk, pt):
                t = sbb.tile([128, BLKF], F32R, name=f"v1b_{img}_{blk}", tag="brblk")
                nc.scalar.activation(t[:], pt[:], AF.Relu, bias=bv1[:, 0:1], scale=1.0)
                pt2 = ps.tile([128, BLKF], F32, name=f"ps_v2_{img}_{blk}", tag="psum")
                nc.tensor.matmul(pt2[0:K, :], wv2[:, 0:K], t[:], start=True, stop=True)
                e1 = sbs.tile([128, BLKF], F32, name=f"var_e_{img}_{blk}", tag="stage")
                nc.scalar.activation(e1[0:K, :], pt2[0:K, :], AF.Exp,
                                     bias=b1x1[0:K, 2:3], scale=1.0)
                st = sbs.tile([128, BLKF], F32, name=f"var_st_{img}_{blk}", tag="stage")
                nc.scalar.activation(st[0:K, :], e1[0:K, :], AF.Ln, bias=1.0)
                nc.sync.dma_start(P["var_o"][img, :, blk * 8:(blk + 1) * 8, :], st[0:K, :])
            conv3(img, P["w_v1"], "v1", spad, 1, evac_v1)


def _decode_phase(nc, tc, P, a_sig, fw_sig):
    import contextlib
    with contextlib.ExitStack() as ctx:
        sbd = ctx.enter_context(tc.tile_pool(name="sbd", bufs=1))

        def big(name):
            return sbd.tile([128, H * W], F32, name=name, tag=name)

        def small(name):
            return sbd.tile([128, 1], F32, name=name, tag=name)

        def small64(name):
            return sbd.tile([128, 64], F32, name=name, tag=name)

        heat4, offx4, offy4 = big("heat4"), big("offx4"), big("offy4")
        e4, scr1, scr2 = big("e4"), big("scr1"), big("scr2")
        g64 = sbd.tile([128, 64], F32, name="g64d", tag="g64d")
        nc.sync.dma_start(g64[:], P["grid"][:])

        for t in (heat4, offx4, offy4):
            nc.vector.memset(t[:], 0.0)
        for b in range(BL):
            nc.sync.dma_start(heat4[32 * b:32 * b + K, :], P["heat_o"][b])
            nc.sync.dma_start(offx4[32 * b:32 * b + K, :], P["off_o"][b, :, 0])
            nc.sync.dma_start(offy4[32 * b:32 * b + K, :], P["off_o"][b, :, 1])

        m = small("m")
        nc.vector.reduce_max(m[:], heat4[:], axis=AX.X)
        nc.sync.dma_start(P["scores_o"][:], m[:])
        negm = small("negm")
        nc.vector.tensor_scalar_mul(negm[:], m[:], -1.0)
        S = small("S")
        nc.scalar.activation(e4[:], heat4[:], AF.Exp, bias=negm[:, 0:1], scale=1.0,
                             accum_out=S[:, 0:1])

        # soft-argmax
        colsum, rowsum, t64 = small64("colsum"), small64("rowsum"), small64("t64")
        nc.vector.reduce_sum(colsum[:], e4[:].rearrange("p (r c) -> p c r", r=64), axis=AX.X)
        nc.vector.reduce_sum(rowsum[:], e4[:].rearrange("p (r c) -> p r c", r=64), axis=AX.X)
        Sx, Sy, Sinv, xc, yc = (small(n) for n in ("Sx", "Sy", "Sinv", "xc", "yc"))
        nc.vector.tensor_tensor(t64[:], colsum[:], g64[:], op=ALU.mult)
        nc.vector.reduce_sum(Sx[:], t64[:], axis=AX.X)
        nc.vector.tensor_tensor(t64[:], rowsum[:], g64[:], op=ALU.mult)
        nc.vector.reduce_sum(Sy[:], t64[:], axis=AX.X)
        nc.vector.reciprocal(Sinv[:], S[:])
        nc.vector.tensor_tensor(xc[:], Sx[:], Sinv[:], op=ALU.mult)
        nc.vector.tensor_tensor(yc[:], Sy[:], Sinv[:], op=ALU.mult)

        # window masks around round(clip(xc)), round(clip(yc))
        xcc, ycc, nxcc, nycc = (small(n) for n in ("xcc", "ycc", "nxcc", "nycc"))
        nc.vector.tensor_scalar(xcc[:], xc[:], 0.0, float(W - 1), op0=ALU.max, op1=ALU.min)
        nc.vector.tensor_scalar(ycc[:], yc[:], 0.0, float(H - 1), op0=ALU.max, op1=ALU.min)
        nc.vector.tensor_scalar_mul(nxcc[:], xcc[:], -1.0)
        nc.vector.tensor_scalar_mul(nycc[:], ycc[:], -1.0)
        adx, ady = small64("adx"), small64("ady")
        nc.scalar.activation(adx[:], g64[:], AF.Abs, bias=nxcc[:, 0:1], scale=1.0)
        nc.scalar.activation(ady[:], g64[:], AF.Abs, bias=nycc[:, 0:1], scale=1.0)
        maskx, masky = small64("maskx"), small64("masky")
        nc.vector.tensor_scalar(maskx[:], adx[:], 2.5, None, op0=ALU.is_lt)
        nc.vector.tensor_scalar(masky[:], ady[:], 2.5, None, op0=ALU.is_lt)

        # refinement: weights ~ e4 * masky(r) * maskx(c)
        nc.vector.tensor_tensor(
            scr1[:].rearrange("p (r c) -> p r c", r=64),
            e4[:].rearrange("p (r c) -> p r c", r=64),
            masky[:].rearrange("p (r o) -> p r o", o=1).broadcast_to([128, 64, 64]),
            op=ALU.mult)
        colsum2, colsum2m = small64("colsum2"), small64("colsum2m")
        nc.vector.reduce_sum(colsum2[:], scr1[:].rearrange("p (r c) -> p c r", r=64), axis=AX.X)
        nc.vector.tensor_tensor(colsum2m[:], colsum2[:], maskx[:], op=ALU.mult)
        den2, rxn, ryn = small("den2"), small("rxn"), small("ryn")
        nc.vector.reduce_sum(den2[:], colsum2m[:], axis=AX.X)
        nc.vector.tensor_tensor(t64[:], colsum2m[:], g64[:], op=ALU.mult)
        nc.vector.reduce_sum(rxn[:], t64[:], axis=AX.X)
        nc.vector.tensor_tensor(
            scr2[:].rearrange("p (r c) -> p r c", r=64),
            e4[:].rearrange("p (r c) -> p r c", r=64),
            maskx[:].rearrange("p (o c) -> p o c", o=1).broadcast_to([128, 64, 64]),
            op=ALU.mult)
        rowsum2, rowsum2m = small64("rowsum2"), small64("rowsum2m")
        nc.vector.reduce_sum(rowsum2[:], scr2[:].rearrange("p (r c) -> p r c", r=64), axis=AX.X)
        nc.vector.tensor_tensor(rowsum2m[:], rowsum2[:], masky[:], op=ALU.mult)
        nc.vector.tensor_tensor(t64[:], rowsum2m[:], g64[:], op=ALU.mult)
        nc.vector.reduce_sum(ryn[:], t64[:], axis=AX.X)
        dinv, rx, ry = small("dinv"), small("rx"), small("ry")
        nc.vector.reciprocal(dinv[:], den2[:])
        nc.vector.tensor_tensor(rx[:], rxn[:], dinv[:], op=ALU.mult)
        nc.vector.tensor_tensor(ry[:], ryn[:], dinv[:], op=ALU.mult)

        # blend coords = a*cg + (1-a)*cl
        a = float(a_sig)
        cx, cy, t1, t2 = (small(n) for n in ("cx", "cy", "t1", "t2"))
        nc.vector.tensor_scalar_mul(t1[:], xc[:], a)
        nc.vector.tensor_scalar_mul(t2[:], rx[:], 1.0 - a)
        nc.vector.tensor_tensor(cx[:], t1[:], t2[:], op=ALU.add)
        nc.vector.tensor_scalar_mul(t1[:], yc[:], a)
        nc.vector.tensor_scalar_mul(t2[:], ry[:], 1.0 - a)
        nc.vector.tensor_tensor(cy[:], t1[:], t2[:], op=ALU.add)

        # bilinear sampling of offsets at (cx, cy): separable tent weights
        ix, iy, nix, niy = (small(n) for n in ("ix", "iy", "nix", "niy"))
        nc.vector.tensor_scalar(ix[:], cx[:], 0.0, float(W - 1), op0=ALU.max, op1=ALU.min)
        nc.vector.tensor_scalar(iy[:], cy[:], 0.0, float(H - 1), op0=ALU.max, op1=ALU.min)
        nc.vector.tensor_scalar_mul(nix[:], ix[:], -1.0)
        nc.vector.tensor_scalar_mul(niy[:], iy[:], -1.0)
        adx2, ady2, wx, wy = (small64(n) for n in ("adx2", "ady2", "wx", "wy"))
        nc.scalar.activation(adx2[:], g64[:], AF.Abs, bias=nix[:, 0:1], scale=1.0)
        nc.scalar.activation(ady2[:], g64[:], AF.Abs, bias=niy[:, 0:1], scale=1.0)
        nc.scalar.activation(wx[:], adx2[:], AF.Relu, bias=1.0, scale=-1.0)
        nc.scalar.activation(wy[:], ady2[:], AF.Relu, bias=1.0, scale=-1.0)

        sampx, sampy = small("sampx"), small("sampy")
        for i, (src, dst) in enumerate(((offx4, sampx), (offy4, sampy))):
            nc.vector.tensor_tensor(
                scr1[:].rearrange("p (r c) -> p r c", r=64),
                src[:].rearrange("p (r c) -> p r c", r=64),
                wy[:].rearrange("p (r o) -> p r o", o=1).broadcast_to([128, 64, 64]),
                op=ALU.mult)
            cs = small64(f"cs{i}")
            nc.vector.reduce_sum(cs[:], scr1[:].rearrange("p (r c) -> p c r", r=64), axis=AX.X)
            nc.vector.tensor_tensor(t64[:], cs[:], wx[:], op=ALU.mult)
            nc.vector.reduce_sum(dst[:], t64[:], axis=AX.X)

        fwv = float(fw_sig)
        coords = sbd.tile([128, 2], F32, name="coords", tag="coords")
        nc.vector.tensor_scalar_mul(t1[:], sampx[:], fwv)
        nc.vector.tensor_tensor(coords[:, 0:1], cx[:], t1[:], op=ALU.add)
        nc.vector.tensor_scalar_mul(t1[:], sampy[:], fwv)
        nc.vector.tensor_tensor(coords[:, 1:2], cy[:], t1[:], op=ALU.add)
        nc.sync.dma_start(P["coords_o"][:], coords[:])


def _sigmoid(v):
    return 1.0 / (1.0 + np.exp(-float(v)))


def prepare_inputs(inputs):
    """Host-side packing shared by kernel() and the test harness."""
    f = lambda n: np.asarray(inputs[n], dtype=np.float32)
    bscale = np.float32(1.0 / np.sqrt(1.0 + BN_EPS))
    sc = lambda g: f(g) * bscale

    perm = list(range(0, 2 * K, 2)) + list(range(1, 2 * K, 2))
    shared = dict(
        w_s1=_pack_conv3(f("w_s1"), sc("g_s1")),
        w_s2=_pack_conv3(f("w_s2"), sc("g_s2")),
        w_h1=_pack_conv3(f("w_h1"), sc("g_h1")),
        w_o1=_pack_conv3(f("w_o1"), sc("g_o1")),
        w_v1=_pack_conv3(f("w_v1"), sc("g_v1")),
        w_h2=_pack_conv1(f("w_h2")),
        w_o2=_pack_conv1(f("w_o2")[perm]),
        w_v2=_pack_conv1(f("w_v2")),
    )
    bias4 = np.zeros((128, 8), np.float32)
    cols = [("b_s1", 0), ("b_s1", 1), ("b_s2", 0), ("b_s2", 1),
            ("b_h1", 0), ("b_h1", 1), ("b_o1", 0), ("b_o1", 1)]
    for j, (nm, t) in enumerate(cols):
        bias4[:, j] = f(nm)[t * 128:(t + 1) * 128]
    shared["bias4"] = bias4
    shared["biasv1"] = f("b_v1").reshape(128, 1).copy()
    bias1x1 = np.zeros((2 * K, 3), np.float32)
    bias1x1[0:K, 0] = f("c_h2")
    bias1x1[0:2 * K, 1] = f("c_o2")[perm]
    bias1x1[0:K, 2] = f("c_v2")
    shared["bias1x1"] = bias1x1
    shared["grid"] = np.tile(np.arange(64, dtype=np.float32), (128, 1))

    x = tf32_round(f("x").reshape(B, C, H * W))
    in_maps = []
    for c in range(NCORE):
        m = dict(shared)
        m["xin"] = np.ascontiguousarray(x[c * BL:(c + 1) * BL])
        in_maps.append(m)
    return in_maps, _sigmoid(inputs["alpha"]), _sigmoid(inputs["fusion"])


def assemble_outputs(results, fw_sig):
    heat = np.concatenate([r["heat_o"] for r in results], 0).reshape(B, K, H, W)
    off = np.concatenate([r["off_o"] for r in results], 0).reshape(B, K, 2, H, W)
    var = np.concatenate([r["var_o"] for r in results], 0).reshape(B, K, H, W)
    coords = np.zeros((B, K, 2), np.float32)
    scores = np.zeros((B, K), np.float32)
    for c, r in enumerate(results):
        cr = r["coords_o"].reshape(BL, 32, 2)
        sr = r["scores_o"].reshape(BL, 32)
        coords[c * BL:(c + 1) * BL] = cr[:, 0:K, :]
        scores[c * BL:(c + 1) * BL] = sr[:, 0:K]
    return heat, off, var, np.float32(fw_sig), coords, scores


def kernel(**inputs):
    in_maps, a_sig, fw_sig = prepare_inputs(inputs)
    nc = build_nc(a_sig, fw_sig, repeat=1)
    nc.compile()
    nc._runner_compiled = True
    res = run_bass_kernel_spmd(nc, in_maps, list(range(NCORE)))
    return assemble_outputs(res.results, fw_sig)
